# revision 21
# baseline (speedup 1.0000x reference)
"""Pointer-generator output layer (scatter_memory) on 8 TRN2 NeuronCores.

Math (per reference):
  context = attn_dist @ enc_output            (only used via W_gen -> associativity)
  p_gen   = sigmoid(attn_dist @ (enc_output @ Wg_c) + x @ Wg_x + b_gen)
  logit   = x @ W_vocab + b_vocab             [16,100,32000]
  out     = log(softmax(logit)*p_gen  (+)scatter  softmax(attn)*(1-p_gen))

Sharding: vocab-parallel over 8 cores (4000 cols each, padded to 4096).
Each core computes, in a [v, r] (vocab-major) orientation (r = b*100+d):
  pass 1: psum = Wsh^T-block @ xT-block accumulation; stash = fp16 exp(logit+bv);
          Z-partials via ones-matmul partition reduction; AllReduce Z.
  attn phase (batch-sharded, 2 batches/core): p_gen, softmax(attn)*(1-p_gen),
          duplicate-id accumulation via id-equality matmul; AllGather.
  pass 2: out = ln(stash) + (log p_gen - log Z) broadcast; store [16,4096,100].
  fixup:  indirect gather of the <=512 scattered columns per batch,
          out_col = log(exp(out_col) + s), indirect scatter back (duplicate ids
          write identical values; out-of-shard ids skipped via bounds check).
Host: shard/cast inputs, unshard + transpose output back to [16,100,32000].
"""

import numpy as np
import ml_dtypes

import concourse.bass as bass
import concourse.tile as tile
from concourse import mybir
from concourse.bass_utils import run_bass_kernel_spmd
from concourse.masks import make_identity

BS, DEC, IN_LEN, HID, VOCAB = 16, 100, 512, 768, 32000
NCORES = 8
VSH = VOCAB // NCORES          # 4000 real vocab per core
VPAD = 4096                    # padded shard width (32 chunks of 128)
VC = VPAD // 128               # 32 v-chunks
KC = HID // 128                # 6 k-chunks
R = BS * DEC                   # 1600 rows
RB = 4                         # r-blocks
RBW = R // RB                  # 400 (= 4 batches) per block
BPC = BS // NCORES             # 2 batches per core (attn phase)
OUTB = VPAD + 64               # out rows per batch incl. dump region (4160)

F32 = mybir.dt.float32
F16 = mybir.dt.float16
BF16 = mybir.dt.bfloat16
I32 = mybir.dt.int32

LN_FLOOR = -45.861420440673828  # ln of the ACT Ln LUT's min input

_CACHE = {}


def _build(ngrp):
    if ngrp in _CACHE:
        return _CACHE[ngrp]
    nc = bass.Bass("TRN2", num_devices=NCORES, debug=False, target_bir_lowering=False)

    xT = nc.dram_tensor("xT", [HID, R], BF16, kind="ExternalInput")
    Wsh = nc.dram_tensor("Wsh", [HID, VPAD], BF16, kind="ExternalInput")
    bvoc = nc.dram_tensor("bvoc", [VPAD], F32, kind="ExternalInput")
    enc_sh = nc.dram_tensor("enc_sh", [BPC, IN_LEN, HID], F32, kind="ExternalInput")
    attn_sh = nc.dram_tensor("attn_sh", [BPC, DEC, IN_LEN], F32, kind="ExternalInput")
    x_sh = nc.dram_tensor("x_sh", [BPC, DEC, HID], F32, kind="ExternalInput")
    ids_sh = nc.dram_tensor("ids_sh", [BPC, IN_LEN], I32, kind="ExternalInput")
    wgen = nc.dram_tensor("wgen", [2, HID], F32, kind="ExternalInput")
    bgen = nc.dram_tensor("bgen", [1, 1], F32, kind="ExternalInput")
    gtab = nc.dram_tensor("gtab", [128, ngrp], I32, kind="ExternalInput")
    stab = nc.dram_tensor("stab", [128, ngrp], I32, kind="ExternalInput")

    out = nc.dram_tensor("out", [BS, OUTB, DEC], F32, kind="ExternalOutput")

    ag_s_in = nc.dram_tensor("ag_s_in", [BPC, IN_LEN, DEC], F32)
    ag_s_out = nc.dram_tensor("ag_s_out", [BS, IN_LEN, DEC], F32, addr_space="Shared")
    ag_p_in = nc.dram_tensor("ag_p_in", [BPC, DEC], F32)
    ag_p_out = nc.dram_tensor("ag_p_out", [BS, DEC], F32, addr_space="Shared")
    ar_z_in = nc.dram_tensor("ar_z_in", [1, R], F32)
    ar_z_out = nc.dram_tensor("ar_z_out", [1, R], F32, addr_space="Shared")
    q_dram = nc.dram_tensor("q_dram", [BPC, IN_LEN], F32)
    c_dram = nc.dram_tensor("c_dram", [1, R], F32)

    def bcast(ap, p=128):
        # replicate a [1, N] DRAM row across p partitions (step-0 partition dim)
        return bass.AP(tensor=ap.tensor, offset=ap.offset, ap=[[0, p]] + list(ap.ap[-1:]))

    groups = [list(range(NCORES))]

    with tile.TileContext(nc) as tc:
        with tc.tile_pool(name="persist", bufs=1) as per:
            stash = per.tile([128, VC, R], F16, tag="stash")
            xT_sb = per.tile([128, KC, R], BF16, tag="xt")
            bvoc_sb = per.tile([128, VC], F32, tag="bvoc")
            cb = per.tile([128, RB, RBW], F32, tag="cb")
            ones16 = per.tile([128, 1], F16, tag="ones16")
            zrow = per.tile([1, R], F32, tag="zrow")

            nc.sync.dma_start(out=xT_sb[:], in_=xT[:].rearrange("(kc p) r -> p kc r", p=128))
            nc.sync.dma_start(
                out=bvoc_sb[:],
                in_=bass.AP(tensor=bvoc, offset=0, ap=[[1, 128], [128, VC]]),
            )
            nc.vector.memset(ones16[:], 1.0)

            # ================= attn / p_gen / dup-sum phase (2 local batches) ====
            with (
                tc.tile_pool(name="aph", bufs=1) as ap_,
                tc.tile_pool(name="aph2", bufs=2) as ap2,
                tc.tile_pool(name="apsum", bufs=2, space="PSUM") as apsum,
                tc.tile_pool(name="apsum2", bufs=2, space="PSUM") as apsum2,
            ):
                ident = ap_.tile([128, 128], F32, tag="ident")
                make_identity(nc, ident[:])
                onesf = ap_.tile([128, 1], F32, tag="onesf")
                nc.vector.memset(onesf[:], 1.0)
                wgc_b = ap_.tile([128, HID], F32, tag="wgc")
                nc.sync.dma_start(out=wgc_b[:], in_=bcast(wgen[0:1, :]))
                wgx_b = ap_.tile([128, HID], F32, tag="wgx")
                nc.sync.dma_start(out=wgx_b[:], in_=bcast(wgen[1:2, :]))
                bgen_b = ap_.tile([128, 1], F32, tag="bgen")
                nc.sync.dma_start(out=bgen_b[:], in_=bcast(bgen[0:1, :]))
                pgen_sb = ap_.tile([128, BPC], F32, tag="pgen")
                s_local = ap_.tile([128, BPC, 4, DEC], F32, tag="sloc")

                for b in range(BPC):
                    # ---- q[b] = enc_output[b] @ Wg_c  -> q_dram[b] ----
                    q_sb = ap2.tile([128, 4], F32, tag="q")
                    for lc in range(4):
                        enc_t = ap2.tile([128, HID], F32, tag="enc")
                        nc.sync.dma_start(out=enc_t[:], in_=enc_sh[b, lc * 128:(lc + 1) * 128, :])
                        prod = ap2.tile([128, HID], F32, tag="prod")
                        nc.vector.tensor_tensor(
                            out=prod[:], in0=enc_t[:], in1=wgc_b[:],
                            op=mybir.AluOpType.mult,
                        )
                        nc.vector.tensor_reduce(
                            out=q_sb[:, lc:lc + 1], in_=prod[:],
                            axis=mybir.AxisListType.X, op=mybir.AluOpType.add,
                        )
                    nc.sync.dma_start(
                        out=bass.AP(tensor=q_dram, offset=b * IN_LEN, ap=[[1, 128], [128, 4]]),
                        in_=q_sb[:],
                    )
                    q_b = ap2.tile([128, IN_LEN], F32, tag="qb")
                    nc.sync.dma_start(out=q_b[:], in_=bcast(q_dram[b:b + 1, :]))

                    # ---- attn softmax + p_gen ----
                    attn_t = ap2.tile([128, IN_LEN], F32, tag="attn")
                    nc.sync.dma_start(out=attn_t[:DEC, :], in_=attn_sh[b])
                    negmax = ap2.tile([128, 1], F32, tag="negmax")
                    nc.vector.tensor_reduce(
                        out=negmax[:DEC], in_=attn_t[:DEC, :], axis=mybir.AxisListType.X,
                        op=mybir.AluOpType.max, negate=True,
                    )
                    expat = ap2.tile([128, IN_LEN], F32, tag="expat")
                    sumexp = ap2.tile([128, 1], F32, tag="sumexp")
                    nc.scalar.activation(
                        out=expat[:DEC, :], in_=attn_t[:DEC, :],
                        func=mybir.ActivationFunctionType.Exp,
                        bias=negmax[:DEC], accum_out=sumexp[:DEC],
                    )
                    t1 = ap2.tile([128, 1], F32, tag="t1")
                    prod3 = ap2.tile([128, IN_LEN], F32, tag="prod3")
                    nc.vector.tensor_tensor(
                        out=prod3[:DEC], in0=attn_t[:DEC, :], in1=q_b[:DEC, :],
                        op=mybir.AluOpType.mult,
                    )
                    nc.vector.tensor_reduce(
                        out=t1[:DEC], in_=prod3[:DEC], axis=mybir.AxisListType.X,
                        op=mybir.AluOpType.add,
                    )
                    x_t = ap2.tile([128, HID], F32, tag="xt2")
                    nc.sync.dma_start(out=x_t[:DEC, :], in_=x_sh[b])
                    t2 = ap2.tile([128, 1], F32, tag="t2")
                    prod4 = ap2.tile([128, HID], F32, tag="prod4")
                    nc.vector.tensor_tensor(
                        out=prod4[:DEC], in0=x_t[:DEC, :], in1=wgx_b[:DEC, :],
                        op=mybir.AluOpType.mult,
                    )
                    nc.vector.tensor_reduce(
                        out=t2[:DEC], in_=prod4[:DEC], axis=mybir.AxisListType.X,
                        op=mybir.AluOpType.add,
                    )
                    t12 = ap2.tile([128, 1], F32, tag="t12")
                    nc.vector.tensor_add(out=t12[:DEC], in0=t1[:DEC], in1=t2[:DEC])
                    tg = ap2.tile([128, 1], F32, tag="tg")
                    nc.vector.tensor_add(out=tg[:DEC], in0=t12[:DEC], in1=bgen_b[:DEC])
                    pg = ap2.tile([128, 1], F32, tag="pg")
                    nc.scalar.activation(
                        out=pg[:DEC], in_=tg[:DEC],
                        func=mybir.ActivationFunctionType.Sigmoid,
                    )
                    # ship pre-sigmoid t; log(p_gen) is computed as -softplus(-t)
                    # to avoid the ACT sigmoid's tiny-output saturation
                    nc.vector.tensor_copy(out=pgen_sb[:DEC, b:b + 1], in_=tg[:DEC])
                    # m = (1 - pg) / sumexp
                    omp = ap2.tile([128, 1], F32, tag="omp")
                    nc.scalar.activation(
                        out=omp[:DEC], in_=pg[:DEC],
                        func=mybir.ActivationFunctionType.Identity,
                        bias=onesf[:DEC], scale=-1.0,
                    )
                    recip = ap2.tile([128, 1], F32, tag="recip")
                    nc.vector.reciprocal(out=recip[:DEC], in_=sumexp[:DEC])
                    m = ap2.tile([128, 1], F32, tag="m")
                    nc.vector.tensor_tensor(
                        out=m[:DEC], in0=omp[:DEC], in1=recip[:DEC], op=mybir.AluOpType.mult,
                    )
                    attn_w = ap2.tile([128, IN_LEN], F32, tag="attnw")
                    nc.vector.tensor_scalar(
                        out=attn_w[:DEC, :], in0=expat[:DEC, :], scalar1=m[:DEC],
                        scalar2=None, op0=mybir.AluOpType.mult,
                    )

                    # ---- transpose attn_w -> awT [l, d] (bf16) ----
                    awT = ap2.tile([128, 4, DEC], F16, tag="awT")
                    for lc in range(4):
                        psT = apsum.tile([128, DEC], F32, tag="psT")
                        nc.tensor.transpose(
                            out=psT[:, :], in_=attn_w[:DEC, lc * 128:(lc + 1) * 128],
                            identity=ident[:DEC, :DEC],
                        )
                        nc.vector.tensor_copy(out=awT[:, lc, :], in_=psT[:])

                    # ---- id equality rows + dup-sum matmuls ----
                    ids_b = ap2.tile([128, IN_LEN], F32, tag="idsb")
                    nc.gpsimd.dma_start(
                        out=ids_b[:],
                        in_=bass.AP(tensor=ids_sh, offset=b * IN_LEN, ap=[[0, 128], [1, IN_LEN]]),
                    )
                    idsT = ap2.tile([128, 4], F32, tag="idsT")
                    nc.gpsimd.dma_start(
                        out=idsT[:],
                        in_=bass.AP(tensor=ids_sh, offset=b * IN_LEN, ap=[[1, 128], [128, 4]]),
                    )
                    Erows = []
                    for kc in range(4):
                        E = ap2.tile([128, IN_LEN], F16, tag=f"E{kc}")
                        nc.vector.tensor_scalar(
                            out=E[:], in0=ids_b[:], scalar1=idsT[:, kc:kc + 1],
                            scalar2=None, op0=mybir.AluOpType.is_equal,
                        )
                        Erows.append(E)
                    for lc in range(4):
                        psd = apsum2.tile([128, DEC], F32, tag="psd")
                        for kc in range(4):
                            nc.tensor.matmul(
                                out=psd[:], lhsT=Erows[kc][:, lc * 128:(lc + 1) * 128],
                                rhs=awT[:, kc, :], start=(kc == 0), stop=(kc == 3),
                            )
                        nc.vector.tensor_copy(out=s_local[:, b, lc, :], in_=psd[:])

                # ---- ship to collectives ----
                nc.sync.dma_start(
                    out=bass.AP(tensor=ag_s_in, offset=0,
                                ap=[[DEC, 128], [IN_LEN * DEC, BPC], [128 * DEC, 4], [1, DEC]]),
                    in_=s_local[:],
                )
                nc.sync.dma_start(
                    out=bass.AP(tensor=ag_p_in, offset=0, ap=[[1, DEC], [DEC, BPC]]),
                    in_=pgen_sb[:DEC, :],
                )
                nc.gpsimd.collective_compute(
                    "AllGather", mybir.AluOpType.bypass, replica_groups=groups,
                    ins=[ag_s_in[:]], outs=[ag_s_out[:]],
                )
                nc.gpsimd.collective_compute(
                    "AllGather", mybir.AluOpType.bypass, replica_groups=groups,
                    ins=[ag_p_in[:]], outs=[ag_p_out[:]],
                )

            # ================= pass 1: logits -> exp stash + Z partials =========
            with (
                tc.tile_pool(name="wp", bufs=3) as wp,
                tc.tile_pool(name="bps", bufs=3, space="PSUM") as bps,
                tc.tile_pool(name="zps", bufs=1, space="PSUM") as zpool,
            ):
                zp = [
                    zpool.tile([1, RBW], F32, tag=f"z{rb}", name=f"z{rb}")
                    for rb in range(RB)
                ]
                for vc in range(VC):
                    wpan = wp.tile([128, KC, 128], BF16, tag="wpan")
                    nc.sync.dma_start(
                        out=wpan[:],
                        in_=Wsh[:, vc * 128:(vc + 1) * 128].rearrange("(kc p) m -> p kc m", p=128),
                    )
                    nv = 128 if vc < VC - 1 else VSH - 128 * (VC - 1)  # 32 on last chunk
                    for rb in range(RB):
                        sl = slice(rb * RBW, (rb + 1) * RBW)
                        ps = bps.tile([128, RBW], F32, tag="ps")
                        for kc in range(KC):
                            nc.tensor.matmul(
                                out=ps[:], lhsT=wpan[:, kc, :], rhs=xT_sb[:, kc, sl],
                                start=(kc == 0), stop=(kc == KC - 1),
                            )
                        nc.scalar.activation(
                            out=stash[:, vc, sl], in_=ps[:],
                            func=mybir.ActivationFunctionType.Exp,
                            bias=bvoc_sb[:, vc:vc + 1],
                        )
                        nc.tensor.matmul(
                            out=zp[rb][:], lhsT=ones16[:nv, :], rhs=stash[:nv, vc, sl],
                            start=(vc == 0), stop=(vc == VC - 1),
                            skip_group_check=True,
                        )
                for rb in range(RB):
                    nc.vector.tensor_copy(out=zrow[:, rb * RBW:(rb + 1) * RBW], in_=zp[rb][:])
                nc.sync.dma_start(out=ar_z_in[:], in_=zrow[:])
                nc.gpsimd.collective_compute(
                    "AllReduce", mybir.AluOpType.add, replica_groups=groups,
                    ins=[ar_z_in[:]], outs=[ar_z_out[:]],
                )

            # ================= c = log(p_gen) - log(Z) ==========================
            with tc.tile_pool(name="cph", bufs=1) as cph:
                zs = cph.tile([1, R], F32, tag="zs")
                nc.sync.dma_start(out=zs[:], in_=ar_z_out[:])
                pga = cph.tile([1, R], F32, tag="pga")
                nc.sync.dma_start(
                    out=pga[:],
                    in_=bass.AP(tensor=ag_p_out, offset=0, ap=[[R, 1], [1, R]]),
                )
                # log p_gen = min(t,0) - ln(1 + e^-|t|)  (all ACT args bounded:
                # the ACT LUT Exp/Sigmoid is garbage for |arg| > ~30)
                abst = cph.tile([1, R], F32, tag="abst")
                nc.scalar.activation(out=abst[:], in_=pga[:],
                                     func=mybir.ActivationFunctionType.Abs)
                eneg = cph.tile([1, R], F32, tag="eneg")
                nc.scalar.activation(out=eneg[:], in_=abst[:],
                                     func=mybir.ActivationFunctionType.Exp,
                                     scale=-1.0)
                sp = cph.tile([1, R], F32, tag="sp")
                nc.scalar.activation(out=sp[:], in_=eneg[:],
                                     func=mybir.ActivationFunctionType.Ln,
                                     bias=1.0)
                mn = cph.tile([1, R], F32, tag="mn")
                nc.vector.tensor_scalar_min(out=mn[:], in0=pga[:], scalar1=0.0)
                logp = cph.tile([1, R], F32, tag="logp")
                nc.vector.tensor_sub(out=logp[:], in0=mn[:], in1=sp[:])
                # scale = p_gen / Z; pass 2 then emits Ln_ACT(stash * scale) so the
                # device Ln LUT's soft floor matches the reference bit-for-bit-ish
                pr = cph.tile([1, R], F32, tag="pr")
                nc.scalar.activation(out=pr[:], in_=logp[:],
                                     func=mybir.ActivationFunctionType.Exp)
                rz = cph.tile([1, R], F32, tag="rz")
                nc.vector.reciprocal(out=rz[:], in_=zs[:])
                crow = cph.tile([1, R], F32, tag="crow")
                nc.vector.tensor_tensor(out=crow[:], in0=pr[:], in1=rz[:],
                                        op=mybir.AluOpType.mult)
                nc.sync.dma_start(out=c_dram[:], in_=crow[:])
                for rb in range(RB):
                    nc.sync.dma_start(
                        out=cb[:, rb, :],
                        in_=bass.AP(tensor=c_dram, offset=rb * RBW, ap=[[0, 128], [1, RBW]]),
                    )

            # ================= pass 2: out = ln(stash) + c ======================
            store_insts = []
            with tc.tile_pool(name="op", bufs=3) as op_:
                for vc in range(VC):
                    ot = op_.tile([128, R], F32, tag="ot")
                    for rb in range(RB):
                        sl = slice(rb * RBW, (rb + 1) * RBW)
                        nc.vector.tensor_tensor(
                            out=ot[:, sl], in0=stash[:, vc, sl], in1=cb[:, rb, :],
                            op=mybir.AluOpType.mult,
                        )
                        nc.scalar.activation(
                            out=ot[:, sl], in_=ot[:, sl],
                            func=mybir.ActivationFunctionType.Ln,
                        )
                    st = nc.sync.dma_start(
                        out=bass.AP(tensor=out, offset=vc * 128 * DEC,
                                    ap=[[DEC, 128], [OUTB * DEC, BS], [1, DEC]]),
                        in_=ot[:].rearrange("p (b d) -> p b d", d=DEC),
                    )
                    store_insts.append(st.ins)

            # ================= fixup: scattered columns =========================
            # host-compacted lanes: lane (p, g) handles one (batch, vocab-id)
            # pair; pad lanes point at the dump region / s row 0.  [128,1]
            # offset tables only -- multi-column tables are broken on HW.
            with tc.tile_pool(name="fx", bufs=1) as fx:
                gtab_t = fx.tile([128, ngrp], I32, tag="gtab")
                gt_dma = nc.gpsimd.dma_start(out=gtab_t[:], in_=gtab[:])
                stab_t = fx.tile([128, ngrp], I32, tag="stab")
                st_dma = nc.gpsimd.dma_start(out=stab_t[:], in_=stab[:])
                g_t = fx.tile([128, ngrp, DEC], F32, tag="gt")
                s_t = fx.tile([128, ngrp, DEC], F32, tag="st")
                fence1 = nc.gpsimd.memset(g_t[:], 0.0)
                for st in store_insts:
                    tile.add_dep_helper(fence1.ins, st, reason="fixup after all base stores")
                tile.add_dep_helper(fence1.ins, gt_dma.ins, reason="gtab ready")
                tile.add_dep_helper(fence1.ins, st_dma.ins, reason="stab ready")
                fence1b = nc.gpsimd.memset(s_t[:], 0.0)
                out_flat = bass.AP(tensor=out, offset=0, ap=[[DEC, BS * OUTB], [1, DEC]])
                ag_flat = bass.AP(tensor=ag_s_out, offset=0, ap=[[DEC, BS * IN_LEN], [1, DEC]])
                gathers = []
                for g in range(ngrp):
                    gth = nc.gpsimd.indirect_dma_start(
                        out=g_t[:, g, :], out_offset=None,
                        in_=out_flat,
                        in_offset=bass.IndirectOffsetOnAxis(ap=gtab_t[:, g:g + 1], axis=0),
                    )
                    gathers.append(gth)
                    nc.gpsimd.indirect_dma_start(
                        out=s_t[:, g, :], out_offset=None,
                        in_=ag_flat,
                        in_offset=bass.IndirectOffsetOnAxis(ap=stab_t[:, g:g + 1], axis=0),
                    )
                nc.scalar.activation(out=g_t[:], in_=g_t[:], func=mybir.ActivationFunctionType.Exp)
                nc.vector.tensor_add(out=g_t[:], in0=g_t[:], in1=s_t[:])
                lnop = nc.scalar.activation(out=g_t[:], in_=g_t[:], func=mybir.ActivationFunctionType.Ln)
                junk = fx.tile([1, 1], F32, tag="junk")
                fence2 = nc.gpsimd.memset(junk[:], 0.0)
                for gth in gathers:
                    tile.add_dep_helper(fence2.ins, gth.ins, reason="gathers done")
                tile.add_dep_helper(fence2.ins, lnop.ins, reason="fix values ready")
                for g in range(ngrp):
                    nc.gpsimd.indirect_dma_start(
                        out=out_flat,
                        out_offset=bass.IndirectOffsetOnAxis(ap=gtab_t[:, g:g + 1], axis=0),
                        in_=g_t[:, g, :], in_offset=None,
                    )

    _fix_multi_waits(nc)
    _CACHE[ngrp] = nc
    return nc


def _fix_multi_waits(nc):
    """walrus's per-struct setupSyncWait accepts very few sync waits on most
    instruction kinds (1 for DMA DIRECT2D and tensor_scalar, unknown for
    others).  Conservatively move all but the last wait of any multi-wait
    instruction onto ENGINE_NOP (InstISA) carriers inserted just before it on
    the same engine -- sequencers execute in order, so the carrier waits gate
    the instruction.  The ISA struct is the one the kernel-tail drain barrier
    uses with 12+ waits, so it is known multi-wait-capable; keep Drain/ISA/
    barrier instructions as-is."""
    skip = (mybir.InstISA, mybir.InstEventSemaphore, mybir.InstAllEngineBarrier)
    njoin = 0
    for f in nc.m.functions:
        for blk in f.blocks:
            insts = list(blk.instructions)
            out = []
            changed = False
            for inst in insts:
                si = getattr(inst, "sync_info", None)
                if (
                    si is not None
                    and not isinstance(inst, skip)
                    and si.on_wait
                    and len(si.on_wait) > 1
                ):
                    w = list(si.on_wait)
                    for k, wait in enumerate(w[:-1]):
                        ev = mybir.InstEventSemaphore(name=f"{inst.name}_wj{k}")
                        ev.engine = inst.engine
                        upd = mybir.SyncUpdate(
                            sync_type="semaphore", id=wait.id,
                            ant_name=wait.ant_name,
                            update_mode="sem-add-imm", update_value=0,
                        )
                        ev.sync_info = mybir.SyncInfo(on_wait=[wait], on_update=[upd])
                        out.append(ev)
                        njoin += 1
                    si.on_wait = w[-1:]
                    changed = True
                out.append(inst)
            if changed:
                blk.instructions = out


def _prep_inputs(x, attn_dist, enc_input, enc_output, W_vocab, b_vocab, W_gen, b_gen):
    x = np.asarray(x, np.float32)
    attn_dist = np.asarray(attn_dist, np.float32)
    enc_input = np.asarray(enc_input, np.int32)
    enc_output = np.asarray(enc_output, np.float32)
    W_vocab = np.asarray(W_vocab, np.float32)
    b_vocab = np.asarray(b_vocab, np.float32)
    W_gen = np.asarray(W_gen, np.float32).reshape(HID + HID, 1)
    b_gen = np.asarray(b_gen, np.float32).reshape(1, 1)

    xT = np.ascontiguousarray(x.reshape(R, HID).T).astype(ml_dtypes.bfloat16)
    wgen2 = np.ascontiguousarray(W_gen[:, 0].reshape(2, HID))

    # fix-lane capacity: max in-shard unique (b,id) count across cores, padded
    ncount = []
    for c in range(NCORES):
        lo = c * VSH
        n = 0
        for b in range(BS):
            ids_b = enc_input[b]
            m = (ids_b >= lo) & (ids_b < lo + VSH)
            n += len(np.unique(ids_b[m]))
        ncount.append(n)
    ngrp = max(1, -(-max(ncount) // 128))
    DUMPROW = VPAD  # b=0 dump region row

    all_pairs = []
    in_maps = []
    for c in range(NCORES):
        lo = c * VSH
        Wp = np.zeros((HID, VPAD), ml_dtypes.bfloat16)
        Wp[:, :VSH] = W_vocab[:, lo:lo + VSH].astype(ml_dtypes.bfloat16)
        bv = np.zeros((VPAD,), np.float32)
        bv[:VSH] = b_vocab[lo:lo + VSH]
        bsl = slice(c * BPC, (c + 1) * BPC)
        # compacted fix lanes for this core: unique (b, id) with id in shard
        pairs = []
        for b in range(BS):
            ids_b = enc_input[b]
            m = (ids_b >= lo) & (ids_b < lo + VSH)
            ls = np.nonzero(m)[0]
            if len(ls) == 0:
                continue
            uniq, first = np.unique(ids_b[ls], return_index=True)
            for v, l in zip(uniq, ls[first]):
                pairs.append((b * OUTB + (v - lo), b * IN_LEN + l))
        all_pairs.append(len(pairs))
        gt = np.full((128, ngrp), DUMPROW, np.int32)
        st = np.zeros((128, ngrp), np.int32)
        for k, (ov, sv) in enumerate(pairs):
            gt[k % 128, k // 128] = ov
            st[k % 128, k // 128] = sv
        in_maps.append({
            "xT": xT,
            "Wsh": Wp,
            "bvoc": bv,
            "enc_sh": np.ascontiguousarray(enc_output[bsl]),
            "attn_sh": np.ascontiguousarray(attn_dist[bsl]),
            "x_sh": np.ascontiguousarray(x[bsl]),
            "ids_sh": np.ascontiguousarray(enc_input[bsl]),
            "wgen": wgen2,
            "bgen": b_gen,
            "gtab": np.ascontiguousarray(gt),
            "stab": np.ascontiguousarray(st),
        })
    return in_maps, ngrp


def kernel(x, attn_dist, enc_input, enc_output, W_vocab, b_vocab, W_gen, b_gen,
           _want_results=False):
    in_maps, ngrp = _prep_inputs(x, attn_dist, enc_input, enc_output,
                                 W_vocab, b_vocab, W_gen, b_gen)
    nc = _build(ngrp)
    res = run_bass_kernel_spmd(nc, in_maps, list(range(NCORES)))
    shards = [res.results[c]["out"][:, :VSH, :] for c in range(NCORES)]  # [16,4000,100]
    full = np.concatenate(shards, axis=1)          # [16, 32000, 100]
    outv = np.ascontiguousarray(full.transpose(0, 2, 1)).astype(np.float32)
    if _want_results:
        return outv, res
    return outv
